# revision 1
# baseline (speedup 1.0000x reference)
"""DecoderAttention (GQA + RoPE + causal) Trainium2 Bass kernel.

Sharding over 8 NeuronCores: core = 4*batch + g where g in [0,4) is the
head-group. Each core computes 4 query heads (o-slice 512g:512g+512 of Wq)
and their shared KV head (slice 128g:128g+128 of Wk/Wv), plus the partial
output projection with the matching 512-column slice of Wo. Host sums the 4
partials per batch.

Per-core dataflow (matmul inputs bf16, f32 PSUM accumulate):
  QT[o,t] = WqT.T @ hsT   (transposed projections; hsT streamed once)
  RoPE applied in [d,t] layout via partition-offset DVE ops
  ST[k,q] = KT_tile.T @ QT  -> exp on ACT (scale folded) -> P[k,q]
  attn_outT[d,q] += V_tile.T @ P ; rowsum[1,q] += ones.T @ P
  normalize at the PSUM->SBUF copy; out[t,h] += ao_tile.T @ WoT

Perf notes vs the original version:
  - All bulk inputs are host-relayouted so each one loads with a single
    dma_start of <=128 descriptors, each >=1.5KB contiguous per partition
    (HWDGE issue cost is ~600ns/instruction; the old 25+ separate loads
    serialized ~15us of descriptor issue on the Sync engine at startup).
  - Loads are split across both HWDGE queues (sync + scalar) with the
    matmul-critical chunks (w k=0, hsT macro-0 quarters) first.
  - Phase B software-pipelines rowsum/AV matmuls at lag 2 inside the score
    loop so the PE never drains while ACT works through the exps.
  - Diagonal (causally half-masked) visits slice the score matmul and exp
    to the live q-range.
  - Phase C of macro m-1 is emitted between the two head-pairs of macro m,
    hiding the reciprocal/broadcast/normalize chain; output staged in one
    [128, 2048] tile per t-tile -> one 1MB DMA (8KB rows) each.
"""
import math
import os
import sys

sys.path.insert(0, "/opt/trn_rl_repo")

import numpy as np
import ml_dtypes

import concourse.bass as bass  # noqa: F401  (registers engines)
import concourse.mybir as mybir
import concourse.tile as tile
from concourse import bacc
from concourse.bass_utils import run_bass_kernel_spmd

B, T, HID = 2, 2048, 2048
H, KVH, D = 16, 4, 128
NH = H // KVH          # q-heads per core = 4
TM = 512               # t/q macro tile
NKT = HID // 128       # 16 contraction k-tiles for projections
NTT = T // 128         # 16 t-tiles
NM = T // TM           # 4 macros
SCALE = 1.0 / math.sqrt(D)
NEG = -1.0e30

f32 = mybir.dt.float32
bf16 = mybir.dt.bfloat16
MMDT = bf16
NP_IN = ml_dtypes.bfloat16
EXP = mybir.ActivationFunctionType.Exp
IDENT = mybir.ActivationFunctionType.Identity
MULT = mybir.AluOpType.mult
ADD = mybir.AluOpType.add

# w / hsT-macro-0 chunk splits: k-tile ranges per chunk (finest first so
# the earliest matmuls unblock as soon as possible)
WCH = [(0, 1), (1, 2), (2, 4), (4, 8), (8, 12), (12, 16)]
HCH = [(0, 1), (1, 2), (2, 4), (4, 8), (8, 12), (12, 16)]

LAST_RESULTS = None  # BassKernelResults of the most recent run (for test.py)

_cache = {}


def _emit(nc, tc, causal):
    ap = {}
    ap["hsTr"] = nc.dram_tensor(
        "hsTr", [NM, 128, NKT, TM], bf16, kind="ExternalInput").ap()
    ap["wqkvTr"] = nc.dram_tensor(
        "wqkvTr", [128, NKT, 768], bf16, kind="ExternalInput").ap()
    ap["woTr"] = nc.dram_tensor(
        "woTr", [128, NH, HID], bf16, kind="ExternalInput").ap()
    ap["bias"] = nc.dram_tensor("bias", [128, 6], f32, kind="ExternalInput").ap()
    ap["cosT"] = nc.dram_tensor("cosT", [D, T], bf16, kind="ExternalInput").ap()
    ap["sinTs"] = nc.dram_tensor("sinTs", [D, T], bf16, kind="ExternalInput").ap()
    ap["dmask2"] = nc.dram_tensor(
        "dmask2", [128, TM + 128], bf16, kind="ExternalInput").ap()
    ap["ones1"] = nc.dram_tensor("ones1", [128, 1], bf16, kind="ExternalInput").ap()
    ap["ident"] = nc.dram_tensor("ident", [128, 128], bf16, kind="ExternalInput").ap()
    if not causal:
        ap["maskT"] = nc.dram_tensor("maskT", [T, T], f32, kind="ExternalInput").ap()
    out_part = nc.dram_tensor("out_part", [T, HID], bf16, kind="ExternalOutput").ap()

    with tc.tile_pool(name="persist", bufs=1) as pper, \
         tc.tile_pool(name="wqkv", bufs=1) as pw, \
         tc.tile_pool(name="wo", bufs=1) as pwo, \
         tc.tile_pool(name="ropecs", bufs=1) as pcs, \
         tc.tile_pool(name="phA", bufs=2) as pa, \
         tc.tile_pool(name="hst", bufs=8) as ph, \
         tc.tile_pool(name="hst0", bufs=1) as ph0, \
         tc.tile_pool(name="ptile", bufs=6) as pp, \
         tc.tile_pool(name="phB", bufs=2) as pb, \
         tc.tile_pool(name="mask", bufs=3) as pm, \
         tc.tile_pool(name="outp", bufs=2) as po:
        qt = [pper.tile([128, T], MMDT, tag=f"qt{h}", name=f"qt{h}") for h in range(NH)]
        kt = pper.tile([128, T], MMDT, tag="kt", name="kt")
        vsb = pper.tile([128, T], MMDT, tag="vsb", name="vsb")
        ao = [pper.tile([128, T], MMDT, tag=f"ao{h}", name=f"ao{h}") for h in range(NH)]

        # ---- startup DMAs -------------------------------------------------
        # hq[m] = list of (k0, k1, tile): hsT k-tiles [k0,k1) of macro m
        hq = {}

        def hq_load(m, eng=None):
            eng = eng or nc.sync
            tiles = []
            for j in range(4):
                ht = ph.tile([128, 4 * TM], MMDT, tag="hst", name="hst")
                eng.dma_start(out=ht[:], in_=ap["hsTr"][m][:, 4 * j:4 * (j + 1), :])
                tiles.append((4 * j, 4 * j + 4, ht))
            return tiles

        w_chunks = [
            pw.tile([128, 768 * (k1 - k0)], MMDT, tag=f"wc{i}", name=f"wc{i}")
            for i, (k0, k1) in enumerate(WCH)]
        hq[0] = [(k0, k1,
                  ph0.tile([128, TM * (k1 - k0)], MMDT, tag=f"hst0{k0}",
                           name="hst0"))
                 for (k0, k1) in HCH]

        # weights stream on the sync queue, macro-0 hsT on the scalar queue:
        # the two HWDGE queues issue in parallel and the DMA engines fair-
        # share, so the k0 feed (wc0 + first h chunk) lands ~3us in.
        for i, (k0, k1) in enumerate(WCH):
            nc.sync.dma_start(out=w_chunks[i][:], in_=ap["wqkvTr"][:, k0:k1, :])
        for (k0, k1, ht) in hq[0]:
            nc.scalar.dma_start(out=ht[:], in_=ap["hsTr"][0][:, k0:k1, :])

        def wsl(k, lo, hi):
            """SBUF slice of weight column range [lo,hi) of k-tile k."""
            for i, (k0, k1) in enumerate(WCH):
                if k0 <= k < k1:
                    off = 768 * (k - k0)
                    return w_chunks[i][:, off + lo:off + hi]
            raise AssertionError

        def hsl_of(m, k):
            """(tile, col slice) holding hsT k-tile k of macro m."""
            for (k0, k1, ht) in hq[m]:
                if k0 <= k < k1:
                    return ht, slice(TM * (k - k0), TM * (k - k0 + 1))
            raise AssertionError

        # scalar HWDGE queue: constants, rope tables, Wo (all off the
        # critical projection path).
        bias_t = pper.tile([128, 6], f32, tag="bias", name="bias")
        nc.scalar.dma_start(out=bias_t[:], in_=ap["bias"][:])
        ones_r = pper.tile([128, 1], MMDT, tag="ones1", name="ones1")
        nc.scalar.dma_start(out=ones_r[:], in_=ap["ones1"][:])
        ident_t = pper.tile([128, 128], bf16, tag="ident", name="ident")
        nc.scalar.dma_start(out=ident_t[:], in_=ap["ident"][:])
        dmask_t = pper.tile([128, TM + 128], MMDT, tag="dmask", name="dmask")
        nc.scalar.dma_start(out=dmask_t[:], in_=ap["dmask2"][:])

        # prefetch hsT macro 1 on the scalar queue (it idles after the
        # macro-0 chunks while sync still streams weights); rope tables +
        # Wo on sync BEHIND the weight feed so they can't steal bandwidth
        # from it (first rope use is ~26us in, Wo's ~120us).
        hq[1] = hq_load(1, nc.scalar)
        cos_t = pcs.tile([128, T], bf16, tag="cosT", name="cosT")
        nc.sync.dma_start(out=cos_t[:], in_=ap["cosT"][:])
        sins_t = pcs.tile([128, T], bf16, tag="sinTs", name="sinTs")
        nc.sync.dma_start(out=sins_t[:], in_=ap["sinTs"][:])
        wo_all = pwo.tile([128, NH * HID], MMDT, tag="wo", name="wo")
        nc.sync.dma_start(out=wo_all[:], in_=ap["woTr"][:])

        # ---------------- Phase A: projections + RoPE + V transpose ---------
        with tc.tile_pool(name="psA", bufs=1, space="PSUM") as psa, \
             tc.tile_pool(name="psAtr", bufs=2, space="PSUM") as psatr:

            def rope(dst, src, tsl):
                # dst = src*cos + rotate_half(src)*sin, in [d, t] layout,
                # all bf16 (qt/kt are bf16 matmul inputs anyway; 16-bit
                # doubles DVE throughput and speeds up GpSimd).
                # sins_t rows d<64 hold +sin[d+64], rows d>=64 hold -sin[d-64],
                # so each mul reads both SBUF inputs at the same base partition
                # (walrus requires equal input base partitions); only the
                # output is partition-shifted.
                tmp = pa.tile([128, TM], bf16, tag="ropetmp", name="ropetmp")
                nc.vector.tensor_tensor(
                    out=tmp[0:64, :], in0=src[64:128, :], in1=sins_t[64:128, tsl], op=MULT)
                nc.vector.tensor_tensor(
                    out=tmp[64:128, :], in0=src[0:64, :], in1=sins_t[0:64, tsl], op=MULT)
                tmp2 = pa.tile([128, TM], bf16, tag="ropetmp2", name="ropetmp2")
                nc.gpsimd.tensor_tensor(
                    out=tmp2[:], in0=src[:], in1=cos_t[:, tsl], op=MULT)
                nc.gpsimd.tensor_tensor(out=dst, in0=tmp2[:], in1=tmp[:], op=ADD)

            vraw_prev = None

            def v_transposes(m, vraw):
                for j in range(4):
                    tt = 4 * m + j
                    tr_ps = psatr.tile([128, 128], bf16, tag="vtr", name="vtr")
                    nc.tensor.transpose(
                        tr_ps[:], vraw[:, 128 * j:128 * (j + 1)], ident_t[:])
                    nc.scalar.copy(vsb[:, 128 * tt:128 * (tt + 1)], tr_ps[:])

            for m in range(NM):
                tsl = slice(TM * m, TM * (m + 1))
                q_ps = [psa.tile([128, TM], f32, tag=f"psq{o}", name=f"psq{o}")
                        for o in range(NH)]
                k_ps = psa.tile([128, TM], f32, tag="psk", name="psk")
                v_ps = psa.tile([128, TM], f32, tag="psv", name="psv")
                for k in range(NKT):
                    h_t, hsl = hsl_of(m, k)
                    st = (k == 0)
                    sp = (k == NKT - 1)
                    for o in range(NH):
                        nc.tensor.matmul(
                            q_ps[o][:], wsl(k, 128 * o, 128 * (o + 1)), h_t[:, hsl],
                            start=st, stop=sp)
                    nc.tensor.matmul(
                        k_ps[:], wsl(k, 512, 640), h_t[:, hsl], start=st, stop=sp)
                    nc.tensor.matmul(
                        v_ps[:], wsl(k, 640, 768), h_t[:, hsl], start=st, stop=sp)
                # prefetch hsT for macro m+2 (reuses macro m's pool slots)
                if m + 2 < NM:
                    hq[m + 2] = hq_load(m + 2)
                last = (m == NM - 1)
                # On the last macro, drain V first and run its transposes
                # immediately: they are the only phase-A PE work left, and
                # doing them before the q/k drains shortens the PSUM-pool
                # handoff barrier into phase B.
                vraw = pa.tile([128, TM], bf16, tag="vraw", name="vraw", bufs=2)
                if last:
                    nc.scalar.activation(vraw[:], v_ps[:], IDENT, bias=bias_t[:, 5:6])
                    v_transposes(m - 1, vraw_prev)
                    v_transposes(m, vraw)
                elif vraw_prev is not None:
                    # previous macro's V transposes: their inputs are long
                    # since ready, so they never stall the PE here.
                    v_transposes(m - 1, vraw_prev)
                # drain the six accumulators on two engines in parallel so
                # the next macro's matmuls get their PSUM banks back quickly
                raws = []
                for o in range(NH):
                    qraw = pa.tile([128, TM], bf16, tag=f"qraw{o}", name=f"qraw{o}")
                    # 2/2 ACT-DVE split normally; 3/3 including kraw on the
                    # last macro so the slowest drain (and with it the
                    # PSUM-pool handoff into phase B) finishes sooner.
                    if o % 2 == 0:
                        nc.scalar.activation(
                            qraw[:], q_ps[o][:], IDENT, bias=bias_t[:, o:o + 1])
                    else:
                        nc.vector.tensor_scalar_add(
                            qraw[:], q_ps[o][:], bias_t[:, o:o + 1])
                    raws.append(qraw)
                kraw = pa.tile([128, TM], bf16, tag="kraw", name="kraw")
                if last:
                    nc.vector.tensor_scalar_add(kraw[:], k_ps[:], bias_t[:, 4:5])
                else:
                    nc.scalar.activation(kraw[:], k_ps[:], IDENT, bias=bias_t[:, 4:5])
                for o in range(NH):
                    rope(qt[o][:, tsl], raws[o], tsl)
                rope(kt[:, tsl], kraw, tsl)
                if not last:
                    nc.scalar.activation(vraw[:], v_ps[:], IDENT, bias=bias_t[:, 5:6])
                vraw_prev = vraw

        # ---------------- Phase B + C: attention + output projection --------
        with tc.tile_pool(name="psSC", bufs=2, space="PSUM") as ps_sc, \
             tc.tile_pool(name="psAV", bufs=1, space="PSUM") as ps_av, \
             tc.tile_pool(name="psRS", bufs=1, space="PSUM") as ps_rs:

            def phase_c(m):
                for j in range(4):
                    tt = 4 * m + j
                    ttsl = slice(128 * tt, 128 * (tt + 1))
                    ot = po.tile([128, HID], bf16, tag="ot", name="ot")
                    for hc in range(4):
                        hsl = slice(512 * hc, 512 * (hc + 1))
                        # rs-tagged banks first: the reciprocal frees them
                        # ~1us earlier than normalize frees the av banks
                        if hc < 2:
                            op_ps = ps_rs.tile([128, TM], f32, tag=f"rs{hc}",
                                               name="opps")
                        else:
                            op_ps = ps_av.tile([128, TM], f32, tag=f"av{hc - 2}",
                                               name="opps")
                        for o in range(4):
                            nc.tensor.matmul(
                                op_ps[:], ao[o][:, ttsl],
                                wo_all[:, HID * o + 512 * hc:HID * o + 512 * (hc + 1)],
                                start=(o == 0), stop=(o == 3))
                        nc.vector.tensor_copy(ot[:, hsl], op_ps[:])
                    nc.sync.dma_start(out=out_part[ttsl, :], in_=ot[:])

            # Flat visit stream across pairs AND macros, software-pipelined
            # at lag 2: the score->exp pipeline never drains at a pair or
            # macro boundary, and each pair's reciprocal/broadcast/normalize
            # chain overlaps the next pair's score visits.
            # Macro order: short m=0 in the middle (its attention is too
            # brief to hide a phase-C block well); end on long m=3 so the
            # final out-DMA tail is just C(3) itself.
            MORD = (1, 2, 0, 3)

            class _Pair:
                def __init__(self, m, pair):
                    self.m, self.pair = m, pair
                    self.nk = 4 * (m + 1) if causal else NTT
                    self.h0, self.h1 = 2 * pair, 2 * pair + 1
                    self.qsl = slice(TM * m, TM * (m + 1))
                    self.av = [ps_av.tile([128, TM], f32, tag=f"av{i}",
                                          name=f"av{i}") for i in range(2)]
                    self.rs = [ps_rs.tile([1, TM], f32, tag=f"rs{i}",
                                          name=f"rs{i}") for i in range(2)]

                def q0(self, kk):
                    # first q column visit kk contributes to (causal):
                    # q_local < 128*jp is entirely masked, never read
                    jp = kk - (self.nk - 4)
                    return 128 * jp if (causal and jp > 0) else 0

            def emit_scores(p, kk):
                ksl = slice(128 * kk, 128 * (kk + 1))
                q0 = p.q0(kk)
                # both heads' score tiles side by side -> one exp pass
                sc = ps_sc.tile([128, 2 * TM], f32, tag="sc", name="sc")
                nc.tensor.matmul(
                    sc[:, q0:TM], kt[:, ksl],
                    qt[p.h0][:, TM * p.m + q0:TM * (p.m + 1)],
                    start=True, stop=True)
                nc.tensor.matmul(
                    sc[:, TM + q0:2 * TM], kt[:, ksl],
                    qt[p.h1][:, TM * p.m + q0:TM * (p.m + 1)],
                    start=True, stop=True)
                pt = pp.tile([128, 2 * TM], MMDT, tag="pt", name="pt")
                if causal:
                    if q0 == 0:
                        nc.scalar.activation(pt[:], sc[:], EXP, scale=SCALE)
                    else:
                        nc.scalar.activation(
                            pt[:, q0:TM], sc[:, q0:TM], EXP, scale=SCALE)
                        nc.scalar.activation(
                            pt[:, TM + q0:2 * TM], sc[:, TM + q0:2 * TM],
                            EXP, scale=SCALE)
                    jp = kk - (p.nk - 4)
                    if jp >= 0:
                        # mask after exp: both heads' 128-wide diagonal
                        # strips in ONE DVE op (fixed per-op cost ~1us
                        # dominates) — the [tri|1s|1s-dead|tri] operand
                        # multiplies head0's valid right part and head1's
                        # never-read left part by 1.0.
                        w0 = 128 * jp
                        nc.vector.tensor_tensor(
                            out=pt[:, w0:TM + w0 + 128],
                            in0=pt[:, w0:TM + w0 + 128],
                            in1=dmask_t[:],
                            op=MULT)
                else:
                    mk = pm.tile([128, TM], f32, tag="mk", name="mk")
                    nc.sync.dma_start(out=mk[:], in_=ap["maskT"][ksl, p.qsl])
                    for base in (0, TM):
                        nc.vector.scalar_tensor_tensor(
                            out=sc[:, base:base + TM],
                            in0=sc[:, base:base + TM],
                            scalar=SCALE, in1=mk[:],
                            op0=MULT, op1=ADD)
                    nc.scalar.activation(pt[:], sc[:], EXP, scale=1.0)
                return pt

            prev_c = [None]

            def drain(p, kk, pt, st, sp):
                q0 = p.q0(kk)
                ksl = slice(128 * kk, 128 * (kk + 1))
                nc.tensor.matmul(p.rs[0][:, q0:TM], ones_r[:],
                                 pt[:, q0:TM], start=st, stop=sp)
                nc.tensor.matmul(p.rs[1][:, q0:TM], ones_r[:],
                                 pt[:, TM + q0:2 * TM], start=st, stop=sp)
                nc.tensor.matmul(p.av[0][:, q0:TM], vsb[:, ksl],
                                 pt[:, q0:TM], start=st, stop=sp)
                nc.tensor.matmul(p.av[1][:, q0:TM], vsb[:, ksl],
                                 pt[:, TM + q0:2 * TM], start=st, stop=sp)
                if not sp:
                    return
                # pair complete: normalize chain (runs on DVE/GpSimd under
                # the next pair's score visits).
                for i, h in ((0, p.h0), (1, p.h1)):
                    inv = pb.tile([1, TM], f32, tag="inv", name="inv")
                    nc.vector.reciprocal_approx_fast(out=inv[:], in_=p.rs[i][:])
                    invb = pb.tile([128, TM], f32, tag=f"invb{i}",
                                   name=f"invb{i}")
                    nc.gpsimd.partition_broadcast(invb[:], inv[:])
                    nc.vector.tensor_tensor(
                        out=ao[h][:, p.qsl], in0=p.av[i][:], in1=invb[:],
                        op=MULT)
                if p.pair == 0 and prev_c[0] is not None:
                    phase_c(prev_c[0])
                if p.pair == 1:
                    prev_c[0] = p.m

            pending = []
            for m in MORD:
                for pair in range(NH // 2):
                    p = _Pair(m, pair)
                    # interleave the 4 diagonal visits among the others so
                    # their ~1.2us DVE mask ops spread across the pair's
                    # span instead of bunching (and serializing) at its end
                    diag = list(range(max(p.nk - 4, 0), p.nk))
                    rest = list(range(0, max(p.nk - 4, 0)))
                    order = []
                    while diag or rest:
                        if rest:
                            order.append(rest.pop(0))
                        if rest:
                            order.append(rest.pop(0))
                        if diag:
                            order.append(diag.pop(0))
                    for idx, kk in enumerate(order):
                        pt = emit_scores(p, kk)
                        pending.append(
                            (p, kk, pt, idx == 0, idx == p.nk - 1))
                        if len(pending) > 3:
                            drain(*pending.pop(0))
            while pending:
                drain(*pending.pop(0))
            phase_c(prev_c[0])


def _build(causal):
    nc = bacc.Bacc("TRN2", target_bir_lowering=False, debug=False, num_devices=8)
    with tile.TileContext(nc) as tc:
        _emit(nc, tc, causal)
    nc.compile()
    return nc


def _canonical_causal_mask():
    neg = np.float32(np.finfo(np.float32).min)
    m = np.where(np.tril(np.ones((T, T), dtype=bool)), np.float32(0.0), neg)
    return m.astype(np.float32)


def kernel(**inputs):
    global LAST_RESULTS
    hs = np.ascontiguousarray(np.asarray(inputs["hidden_states"], dtype=np.float32))
    cos = np.asarray(inputs["cos"], dtype=np.float32)
    sin = np.asarray(inputs["sin"], dtype=np.float32)
    mask = np.asarray(inputs["attention_mask"], dtype=np.float32)
    Wq = np.asarray(inputs["Wq"], dtype=np.float32)
    Wk = np.asarray(inputs["Wk"], dtype=np.float32)
    Wv = np.asarray(inputs["Wv"], dtype=np.float32)
    Wo = np.asarray(inputs["Wo"], dtype=np.float32)
    bq = np.asarray(inputs["bq"], dtype=np.float32)
    bk = np.asarray(inputs["bk"], dtype=np.float32)
    bv = np.asarray(inputs["bv"], dtype=np.float32)

    causal = bool(np.array_equal(mask[0, 0], _canonical_causal_mask()))

    key = (causal,)
    if key not in _cache:
        _cache[key] = _build(causal)
    nc = _cache[key]

    tri01 = (np.arange(128)[:, None] <= np.arange(128)[None, :]).astype(NP_IN)
    dmask2 = np.concatenate(
        [tri01, np.ones((128, TM - 128), dtype=NP_IN), tri01], axis=1)
    ident = np.eye(128, dtype=np.float32)
    ones1 = np.ones((128, 1), dtype=NP_IN)
    if not causal:
        maskT = np.ascontiguousarray(mask[0, 0].T)

    in_maps = []
    for c in range(8):
        b, g = divmod(c, 4)
        sl_q = slice(512 * g, 512 * (g + 1))
        sl_kv = slice(128 * g, 128 * (g + 1))
        sinT = np.ascontiguousarray(sin[b].T)  # [D, T]
        # row d<64: +sin[d+64] (consumed at base partition 0 writing rows 64:128)
        # row d>=64: -sin[d-64] (consumed at base partition 64 writing rows 0:64)
        sinTs = np.concatenate([sinT[64:128], -sinT[0:64]], axis=0)
        bias = np.zeros((128, 6), dtype=np.float32)
        bias[:, 0:4] = bq[sl_q].reshape(4, 128).T
        bias[:, 4] = bk[sl_kv]
        bias[:, 5] = bv[sl_kv]
        # [p, k, c] / [m, p, k, c] layouts: each DMA descriptor covers a
        # contiguous >=1.5KB run per partition.
        hsT = hs[b].T.astype(NP_IN)                            # [HID, T]
        hsTr = np.ascontiguousarray(
            hsT.reshape(NKT, 128, NM, TM).transpose(2, 1, 0, 3))
        wqkvT = np.concatenate(
            [Wq[sl_q], Wk[sl_kv], Wv[sl_kv]], axis=0).T.astype(NP_IN)  # [HID, 768]
        wqkvTr = np.ascontiguousarray(
            wqkvT.reshape(NKT, 128, 768).transpose(1, 0, 2))
        woT = Wo[:, sl_q].T.astype(NP_IN)                      # [512, HID]
        woTr = np.ascontiguousarray(woT.reshape(NH, 128, HID).transpose(1, 0, 2))
        m = {
            "hsTr": hsTr,
            "wqkvTr": wqkvTr,
            "woTr": woTr,
            "bias": bias,
            "cosT": np.ascontiguousarray(cos[b].T).astype(NP_IN),
            "sinTs": np.ascontiguousarray(sinTs).astype(NP_IN),
            "dmask2": dmask2,
            "ones1": ones1,
            "ident": ident.astype(NP_IN),
        }
        if not causal:
            m["maskT"] = maskT
        in_maps.append(m)

    trace = os.environ.get("KERNEL_TRACE", "0") == "1"
    res = run_bass_kernel_spmd(nc, in_maps, list(range(8)), trace=trace)
    LAST_RESULTS = res

    out = np.empty((B, T, HID), dtype=np.float32)
    for b in range(B):
        acc = np.asarray(res.results[4 * b]["out_part"], dtype=np.float32)
        for g in range(1, 4):
            acc = acc + np.asarray(
                res.results[4 * b + g]["out_part"], dtype=np.float32)
        out[b] = acc
    return out



# revision 9
# speedup vs baseline: 1.1301x; 1.1301x over previous
"""DecoderAttention (GQA + RoPE + causal) Trainium2 Bass kernel.

Sharding over 8 NeuronCores: core = 4*batch + g where g in [0,4) is the
head-group. Each core computes 4 query heads (o-slice 512g:512g+512 of Wq)
and their shared KV head (slice 128g:128g+128 of Wk/Wv), plus the partial
output projection with the matching 512-column slice of Wo. Host sums the 4
partials per batch.

Precision plan (gate: rel err < 2e-2): fp8 noise enters the output scaled by
1/sqrt(N_eff) of the softmax averaging, so causal rows with few keys (t<512,
macro 0) are the only ones that can't absorb it. Macro 0 therefore runs the
whole chain in bf16 (projections, P, AV, rowsum); macros 1-3 run fp8
DoubleRow. Simulated: hybrid 6.0e-3 vs 4.1e-2 all-fp8 vs 3.1e-3 all-bf16.

Per-core dataflow:
  Phase A (macro order 1,2,3,0): QT[o,t] = WqT.T @ hsT. Macros 1-3 in fp8
    DoubleRow (two 128-k-tiles per PE instruction, 2x; W prescaled x16 on
    host, rescaled in the PSUM drain); macro 0 in bf16 from separately
    streamed bf16 W/hsT chunks (their DMAs hide under macros 1-3). RoPE in
    [d,t] layout, 4 q-heads batched per DVE op via stride-0 broadcast APs.
  Phase B: ST[k,q] = KT_tile.T @ QT in bf16 (contraction is only D=128) ->
    exp on ACT (scale folded, -2 shift that cancels in normalization) into
    k-tile-PAIR P tiles [128, 2, 2*TM]; AV + rowsum as fp8 DoubleRow over
    pairs (rowsum stationary padded to 32 cols: M=1 DoubleRow ldweights is
    invalid ISA). Macro-0 pairs use bf16 P/V and per-k-tile matmuls.
    Diagonal pairs extend the odd half's scores down to the even half's q0
    and mask both halves with two windowed DVE multiplies (dmask2 operand
    [tri|1s|tri]; dmask3 [0s|tri|1s|0s|tri] starting 128 early).
  Phase C: out[t,h] += ao_tile.T @ WoT in bf16 (fp8 here hits the output
    directly), one [128,2048] stage tile -> one 1MB DMA per t-tile.
"""
import math
import os
import sys

sys.path.insert(0, "/opt/trn_rl_repo")

import numpy as np
import ml_dtypes

import concourse.bass as bass  # noqa: F401  (registers engines)
import concourse.mybir as mybir
import concourse.tile as tile
from concourse import bacc
from concourse.bass_utils import run_bass_kernel_spmd

B, T, HID = 2, 2048, 2048
H, KVH, D = 16, 4, 128
NH = H // KVH          # q-heads per core = 4
TM = 512               # t/q macro tile
NKT = HID // 128       # 16 contraction k-tiles for projections
NTT = T // 128         # 16 t-tiles
NM = T // TM           # 4 macros
SCALE = 1.0 / math.sqrt(D)
ESHIFT = -2.0          # exp(s*scale + ESHIFT): keeps fp8 P well under 240
SW = 16.0              # host prescale on Wq/Wk/Wv before fp8 quantization
INV_SW = 1.0 / SW

f32 = mybir.dt.float32
bf16 = mybir.dt.bfloat16
fp8 = mybir.dt.float8e4
MMDT = bf16
NP_IN = ml_dtypes.bfloat16
NP8 = ml_dtypes.float8_e4m3
EXP = mybir.ActivationFunctionType.Exp
IDENT = mybir.ActivationFunctionType.Identity
MULT = mybir.AluOpType.mult
ADD = mybir.AluOpType.add
DR = mybir.MatmulPerfMode.DoubleRow

# fp8 w / first-fp8-macro hsT chunk splits: k-tile ranges per chunk (finest
# first so the earliest matmuls unblock as soon as possible). Even-aligned so
# DoubleRow k-tile pairs never straddle a chunk boundary.
WCH = [(0, 2), (2, 4), (4, 8), (8, 12), (12, 16)]
HCH = [(0, 2), (2, 4), (4, 8), (8, 12), (12, 16)]
# bf16 macro-0 streams in 4-k-tile chunks through small double-buffered pools
BCH = [(0, 4), (4, 8), (8, 12), (12, 16)]

MAORD = (1, 2, 3, 0)   # phase A macro order: bf16 macro 0 last so its
                       # bf16 W/hsT DMAs hide under the fp8 macros

LAST_RESULTS = None  # BassKernelResults of the most recent run (for test.py)

_cache = {}


def _emit(nc, tc, causal):
    ap = {}
    ap["hsTr"] = nc.dram_tensor(
        "hsTr", [NM, 128, NKT, TM], fp8, kind="ExternalInput").ap()
    ap["hsTr16"] = nc.dram_tensor(
        "hsTr16", [128, NKT, TM], bf16, kind="ExternalInput").ap()
    ap["wqkvTr"] = nc.dram_tensor(
        "wqkvTr", [128, NKT, 768], fp8, kind="ExternalInput").ap()
    ap["wqkvTr16"] = nc.dram_tensor(
        "wqkvTr16", [128, NKT, 768], bf16, kind="ExternalInput").ap()
    ap["woTr"] = nc.dram_tensor(
        "woTr", [128, NH, HID], bf16, kind="ExternalInput").ap()
    ap["bias"] = nc.dram_tensor("bias", [128, 13], f32, kind="ExternalInput").ap()
    ap["cosT"] = nc.dram_tensor("cosT", [D, T], bf16, kind="ExternalInput").ap()
    ap["sinTs"] = nc.dram_tensor("sinTs", [D, T], bf16, kind="ExternalInput").ap()
    ap["dmask2"] = nc.dram_tensor(
        "dmask2", [128, TM + 128], fp8, kind="ExternalInput").ap()
    ap["dmask3"] = nc.dram_tensor(
        "dmask3", [128, TM + 256], fp8, kind="ExternalInput").ap()
    ap["dmask2b"] = nc.dram_tensor(
        "dmask2b", [128, TM + 128], bf16, kind="ExternalInput").ap()
    ap["dmask3b"] = nc.dram_tensor(
        "dmask3b", [128, TM + 256], bf16, kind="ExternalInput").ap()
    ap["ident"] = nc.dram_tensor("ident", [128, 128], bf16, kind="ExternalInput").ap()
    if not causal:
        ap["maskT"] = nc.dram_tensor("maskT", [T, T], f32, kind="ExternalInput").ap()
    out_part = nc.dram_tensor("out_part", [T, HID], bf16, kind="ExternalOutput").ap()

    with tc.tile_pool(name="persist", bufs=1) as pper, \
         tc.tile_pool(name="wqkv", bufs=1) as pw, \
         tc.tile_pool(name="w16", bufs=2) as pw16, \
         tc.tile_pool(name="wo", bufs=1) as pwo, \
         tc.tile_pool(name="ropecs", bufs=1) as pcs, \
         tc.tile_pool(name="phA", bufs=2) as pa, \
         tc.tile_pool(name="hst", bufs=6) as ph, \
         tc.tile_pool(name="hst1", bufs=1) as ph1, \
         tc.tile_pool(name="hst16", bufs=2) as ph16, \
         tc.tile_pool(name="ptile", bufs=3) as pp, \
         tc.tile_pool(name="ptb", bufs=3) as ppb, \
         tc.tile_pool(name="phB", bufs=2) as pb, \
         tc.tile_pool(name="mask", bufs=3) as pm, \
         tc.tile_pool(name="outp", bufs=2) as po:
        qt = pper.tile([128, NH, T], MMDT, tag="qt", name="qt")
        kt = pper.tile([128, T], MMDT, tag="kt", name="kt")
        vsb = pper.tile([128, NTT, 128], fp8, tag="vsb", name="vsb")
        vsb16 = pper.tile([128, 4, 128], bf16, tag="vsb16", name="vsb16")
        ao = [pper.tile([128, T], MMDT, tag=f"ao{h}", name=f"ao{h}") for h in range(NH)]
        ones2 = pper.tile([128, 2, 32], fp8, tag="ones2", name="ones2")
        nc.gpsimd.memset(ones2[:], 1.0)
        ones1b = pper.tile([128, 1], bf16, tag="ones1b", name="ones1b")
        nc.gpsimd.memset(ones1b[:], 1.0)

        # ---- startup DMAs -------------------------------------------------
        # hq[m] = list of (k0, k1, tile): hsT k-tiles [k0,k1) of macro m
        hq = {}

        def hq_load(m, eng=None):
            eng = eng or nc.sync
            tiles = []
            for j in range(4):
                ht = ph.tile([128, 4, TM], fp8, tag="hst", name="hst")
                eng.dma_start(out=ht[:], in_=ap["hsTr"][m][:, 4 * j:4 * (j + 1), :])
                tiles.append((4 * j, 4 * j + 4, ht))
            return tiles

        w_chunks = [
            pw.tile([128, k1 - k0, 768], fp8, tag=f"wc{i}", name=f"wc{i}")
            for i, (k0, k1) in enumerate(WCH)]
        # macro 1 is first: its hsT streams in fine chunks on the scalar
        # queue so the k0 feed lands early
        hq[1] = [(k0, k1,
                  ph1.tile([128, k1 - k0, TM], fp8, tag=f"hst1{k0}",
                           name="hst1"))
                 for (k0, k1) in HCH]
        for i, (k0, k1) in enumerate(WCH):
            nc.sync.dma_start(out=w_chunks[i][:], in_=ap["wqkvTr"][:, k0:k1, :])
        for (k0, k1, ht) in hq[1]:
            nc.scalar.dma_start(out=ht[:], in_=ap["hsTr"][1][:, k0:k1, :])

        def wslp(k, lo, hi):
            """DoubleRow lhsT [128, 2, hi-lo] of weight cols [lo,hi) for
            k-tile pair (k, k+1)."""
            for i, (k0, k1) in enumerate(WCH):
                if k0 <= k < k1:
                    return w_chunks[i][:, k - k0:k - k0 + 2, lo:hi]
            raise AssertionError

        def hslp(m, k):
            """DoubleRow rhs [128, 2, TM] holding hsT k-tile pair (k, k+1)."""
            for (k0, k1, ht) in hq[m]:
                if k0 <= k < k1:
                    return ht[:, k - k0:k - k0 + 2, :]
            raise AssertionError

        # scalar HWDGE queue: constants, rope tables (off the critical
        # projection path).
        bias_t = pper.tile([128, 13], f32, tag="bias", name="bias")
        nc.scalar.dma_start(out=bias_t[:], in_=ap["bias"][:])
        ident_t = pper.tile([128, 128], bf16, tag="ident", name="ident")
        nc.scalar.dma_start(out=ident_t[:], in_=ap["ident"][:])
        dmask_t = pper.tile([128, TM + 128], fp8, tag="dmask", name="dmask")
        nc.scalar.dma_start(out=dmask_t[:], in_=ap["dmask2"][:])
        dmask3_t = pper.tile([128, TM + 256], fp8, tag="dmask3", name="dmask3")
        nc.scalar.dma_start(out=dmask3_t[:], in_=ap["dmask3"][:])
        dmaskb_t = pper.tile([128, TM + 128], bf16, tag="dmaskb", name="dmaskb")
        nc.scalar.dma_start(out=dmaskb_t[:], in_=ap["dmask2b"][:])
        dmask3b_t = pper.tile([128, TM + 256], bf16, tag="dmask3b", name="dmask3b")
        nc.scalar.dma_start(out=dmask3b_t[:], in_=ap["dmask3b"][:])

        # prefetch hsT macro 2 on the scalar queue; rope tables on sync
        # BEHIND the fp8 weight feed so they can't steal bandwidth from it.
        hq[2] = hq_load(2, nc.scalar)
        cos_t = pcs.tile([128, T], bf16, tag="cosT", name="cosT")
        nc.sync.dma_start(out=cos_t[:], in_=ap["cosT"][:])
        sins_t = pcs.tile([128, T], bf16, tag="sinTs", name="sinTs")
        nc.sync.dma_start(out=sins_t[:], in_=ap["sinTs"][:])

        # bf16 macro-0 W / hsT: first chunks issued now (needed only ~35us
        # in, after macros 1-3), the rest from inside the loop.
        w16_t = {}
        h16_t = {}

        def w16_load(ci):
            k0, k1 = BCH[ci]
            wt = pw16.tile([128, 4, 768], bf16, tag="w16", name="w16")
            nc.sync.dma_start(out=wt[:], in_=ap["wqkvTr16"][:, k0:k1, :])
            w16_t[ci] = wt

        def h16_load(ci):
            k0, k1 = BCH[ci]
            ht = ph16.tile([128, 4, TM], bf16, tag="h16", name="h16")
            nc.scalar.dma_start(out=ht[:], in_=ap["hsTr16"][:, k0:k1, :])
            h16_t[ci] = ht

        w16_load(0)
        h16_load(0)
        wo_all = pwo.tile([128, NH * HID], MMDT, tag="wo", name="wo")
        nc.sync.dma_start(out=wo_all[:], in_=ap["woTr"][:])
        w16_load(1)
        h16_load(1)

        # ---------------- Phase A: projections + RoPE + V transpose ---------
        with tc.tile_pool(name="psA", bufs=1, space="PSUM") as psa, \
             tc.tile_pool(name="psAtr", bufs=2, space="PSUM") as psatr:

            def rope_k(dst, src, tsl):
                # dst = src*cos + rotate_half(src)*sin, in [d, t] layout.
                # sins_t rows d<64 hold +sin[d+64], rows d>=64 hold -sin[d-64],
                # so each mul reads both SBUF inputs at the same base partition
                # (walrus requires equal input base partitions); only the
                # output is partition-shifted.
                tmp = pa.tile([128, TM], bf16, tag="ropektmp", name="ropektmp")
                nc.vector.tensor_tensor(
                    out=tmp[0:64, :], in0=src[64:128, :], in1=sins_t[64:128, tsl], op=MULT)
                nc.vector.tensor_tensor(
                    out=tmp[64:128, :], in0=src[0:64, :], in1=sins_t[0:64, tsl], op=MULT)
                tmp2 = pa.tile([128, TM], bf16, tag="ropektmp2", name="ropektmp2")
                nc.gpsimd.tensor_tensor(
                    out=tmp2[:], in0=src[:], in1=cos_t[:, tsl], op=MULT)
                nc.gpsimd.tensor_tensor(out=dst, in0=tmp2[:], in1=tmp[:], op=ADD)

            def rope_q4(qraw4, tsl):
                # all 4 q-heads in one pass; cos/sin broadcast along the
                # head dim with stride-0 APs so the DVE's fixed per-op cost
                # is paid once per step, not once per head.
                s_hi = sins_t[64:128, tsl].unsqueeze(1).broadcast_to([64, NH, TM])
                s_lo = sins_t[0:64, tsl].unsqueeze(1).broadcast_to([64, NH, TM])
                c_bc = cos_t[:, tsl].unsqueeze(1).broadcast_to([128, NH, TM])
                tmp = pa.tile([128, NH, TM], bf16, tag="ropetmp", name="ropetmp")
                nc.vector.tensor_tensor(
                    out=tmp[0:64, :, :], in0=qraw4[64:128, :, :], in1=s_hi, op=MULT)
                nc.vector.tensor_tensor(
                    out=tmp[64:128, :, :], in0=qraw4[0:64, :, :], in1=s_lo, op=MULT)
                tmp2 = pa.tile([128, NH, TM], bf16, tag="ropetmp2", name="ropetmp2")
                nc.vector.tensor_tensor(
                    out=tmp2[:], in0=qraw4[:], in1=c_bc, op=MULT)
                nc.gpsimd.tensor_tensor(
                    out=qt[:, :, tsl], in0=tmp2[:], in1=tmp[:], op=ADD)

            vprev = [None, None]  # (macro, vraw) of the previous macro

            def v_transposes(m, vraw):
                for j in range(4):
                    tt = 4 * m + j
                    tr_ps = psatr.tile([128, 128], bf16, tag="vtr", name="vtr")
                    nc.tensor.transpose(
                        tr_ps[:], vraw[:, 128 * j:128 * (j + 1)], ident_t[:])
                    nc.scalar.copy(vsb[:, tt, :], tr_ps[:])
                    if m == 0:
                        nc.vector.tensor_copy(vsb16[:, j, :], tr_ps[:])

            for mi, m in enumerate(MAORD):
                tsl = slice(TM * m, TM * (m + 1))
                bfm = (m == 0)
                q_ps = [psa.tile([128, TM], f32, tag=f"psq{o}", name=f"psq{o}")
                        for o in range(NH)]
                k_ps = psa.tile([128, TM], f32, tag="psk", name="psk")
                v_ps = psa.tile([128, TM], f32, tag="psv", name="psv")
                if bfm:
                    # bf16 macro 0: per-k-tile matmuls from streamed chunks
                    for ci, (k0, k1) in enumerate(BCH):
                        wt, ht = w16_t[ci], h16_t[ci]
                        for k in range(k0, k1):
                            st = (k == 0)
                            sp = (k == NKT - 1)
                            j = k - k0
                            for o in range(NH):
                                nc.tensor.matmul(
                                    q_ps[o][:], wt[:, j, 128 * o:128 * (o + 1)],
                                    ht[:, j, :], start=st, stop=sp)
                            nc.tensor.matmul(
                                k_ps[:], wt[:, j, 512:640], ht[:, j, :],
                                start=st, stop=sp)
                            nc.tensor.matmul(
                                v_ps[:], wt[:, j, 640:768], ht[:, j, :],
                                start=st, stop=sp)
                        # prefetch the chunk after next now that this one's
                        # readers exist (slot reuse must see them)
                        if ci + 2 < len(BCH):
                            w16_load(ci + 2)
                            h16_load(ci + 2)
                else:
                    for kp in range(0, NKT, 2):
                        h_ap = hslp(m, kp)
                        st = (kp == 0)
                        sp = (kp == NKT - 2)
                        for o in range(NH):
                            nc.tensor.matmul(
                                q_ps[o][:], wslp(kp, 128 * o, 128 * (o + 1)),
                                h_ap, start=st, stop=sp, perf_mode=DR)
                        nc.tensor.matmul(
                            k_ps[:], wslp(kp, 512, 640), h_ap, start=st,
                            stop=sp, perf_mode=DR)
                        nc.tensor.matmul(
                            v_ps[:], wslp(kp, 640, 768), h_ap, start=st,
                            stop=sp, perf_mode=DR)
                # prefetch the NEXT fp8 macro's hsT (distance 1: the 1MB DMA
                # takes ~3us against ~11us of macro compute)
                if mi + 1 < len(MAORD) and MAORD[mi + 1] != 0 \
                        and MAORD[mi + 1] not in hq:
                    hq[MAORD[mi + 1]] = hq_load(MAORD[mi + 1])
                last = (mi == len(MAORD) - 1)
                # drain scale: fp8 macros carry the x16 W prescale
                dsc = 1.0 if bfm else INV_SW
                bco = 0 if bfm else 6  # DVE-path bias column offset (b*SW)
                vraw = pa.tile([128, TM], bf16, tag="vraw", name="vraw", bufs=2)
                if last:
                    nc.scalar.activation(vraw[:], v_ps[:], IDENT,
                                         bias=bias_t[:, 5:6], scale=dsc)
                    v_transposes(vprev[0], vprev[1])
                    v_transposes(m, vraw)
                elif vprev[1] is not None:
                    # previous macro's V transposes: their inputs are long
                    # since ready, so they never stall the PE here.
                    v_transposes(vprev[0], vprev[1])
                # drain the six accumulators on two engines in parallel so
                # the next macro's matmuls get their PSUM banks back quickly.
                # ACT path: ident(ps*dsc + b); DVE path: (ps + b/dsc)*dsc.
                qraw4 = pa.tile([128, NH, TM], bf16, tag="qraw4", name="qraw4")
                for o in range(NH):
                    if o % 2 == 0:
                        nc.scalar.activation(
                            qraw4[:, o, :], q_ps[o][:], IDENT,
                            bias=bias_t[:, o:o + 1], scale=dsc)
                    else:
                        nc.vector.tensor_scalar(
                            qraw4[:, o, :], q_ps[o][:],
                            bias_t[:, bco + o:bco + o + 1], dsc,
                            op0=ADD, op1=MULT)
                kraw = pa.tile([128, TM], bf16, tag="kraw", name="kraw")
                if last:
                    nc.vector.tensor_scalar(
                        kraw[:], k_ps[:], bias_t[:, bco + 4:bco + 5], dsc,
                        op0=ADD, op1=MULT)
                else:
                    nc.scalar.activation(kraw[:], k_ps[:], IDENT,
                                         bias=bias_t[:, 4:5], scale=dsc)
                rope_q4(qraw4, tsl)
                rope_k(kt[:, tsl], kraw, tsl)
                if not last:
                    nc.scalar.activation(vraw[:], v_ps[:], IDENT,
                                         bias=bias_t[:, 5:6], scale=dsc)
                vprev = [m, vraw]

        # ---------------- Phase B + C: attention + output projection --------
        with tc.tile_pool(name="psSC", bufs=2, space="PSUM") as ps_sc, \
             tc.tile_pool(name="psAV", bufs=1, space="PSUM") as ps_av, \
             tc.tile_pool(name="psRS", bufs=1, space="PSUM") as ps_rs:

            def phase_c(m):
                for j in range(4):
                    tt = 4 * m + j
                    ttsl = slice(128 * tt, 128 * (tt + 1))
                    ot = po.tile([128, HID], bf16, tag="ot", name="ot")
                    for hc in range(4):
                        hsl = slice(512 * hc, 512 * (hc + 1))
                        # rs-tagged banks first: the reciprocal frees them
                        # earlier than normalize frees the av banks
                        if hc < 2:
                            op_ps = ps_rs.tile([128, TM], f32, tag=f"rs{hc}",
                                               name="opps")
                        else:
                            op_ps = ps_av.tile([128, TM], f32, tag=f"av{hc - 2}",
                                               name="opps")
                        for o in range(4):
                            nc.tensor.matmul(
                                op_ps[:], ao[o][:, ttsl],
                                wo_all[:, HID * o + 512 * hc:HID * o + 512 * (hc + 1)],
                                start=(o == 0), stop=(o == 3))
                        nc.vector.tensor_copy(ot[:, hsl], op_ps[:])
                    nc.sync.dma_start(out=out_part[ttsl, :], in_=ot[:])

            # Flat stream of k-tile-PAIR visits across head-pairs AND macros,
            # software-pipelined at lag 2. Macro order: short m=0 in the
            # middle (also after phase A's last macro, which is 0); end on
            # long m=3 so the final out-DMA tail is just C(3) itself.
            MORD = (1, 2, 0, 3)

            class _Pair:
                def __init__(self, m, pair):
                    self.m, self.pair = m, pair
                    self.nk = 4 * (m + 1) if causal else NTT
                    self.bf = (m == 0) and causal  # bf16 island (low N_eff)
                    self.h0, self.h1 = 2 * pair, 2 * pair + 1
                    self.qsl = slice(TM * m, TM * (m + 1))
                    self.av = [ps_av.tile([128, TM], f32, tag=f"av{i}",
                                          name=f"av{i}") for i in range(2)]
                    self.rs = [ps_rs.tile([32, TM], f32, tag=f"rs{i}",
                                          name=f"rs{i}") for i in range(2)]

                def q0p(self, kk0):
                    # first q column the PAIR (kk0, kk0+1) contributes to
                    # (causal): the even half's own q0; the odd half's score
                    # range is extended down to it and mask-zeroed.
                    jp = kk0 - (self.nk - 4)
                    return 128 * jp if (causal and jp > 0) else 0

            def emit_scores_pair(p, kk0):
                q0 = p.q0p(kk0)
                # pt holds P for both k-tiles of the pair and both heads:
                # [128, i (pair half), h*TM + q]
                if p.bf:
                    pt = ppb.tile([128, 2, 2 * TM], bf16, tag="ptb", name="ptb")
                    dm2, dm3 = dmaskb_t, dmask3b_t
                else:
                    pt = pp.tile([128, 2, 2 * TM], fp8, tag="pt", name="pt")
                    dm2, dm3 = dmask_t, dmask3_t
                for i in range(2):
                    kk = kk0 + i
                    ksl = slice(128 * kk, 128 * (kk + 1))
                    sc = ps_sc.tile([128, 2 * TM], f32, tag="sc", name="sc")
                    nc.tensor.matmul(
                        sc[:, q0:TM], kt[:, ksl],
                        qt[:, p.h0, TM * p.m + q0:TM * (p.m + 1)],
                        start=True, stop=True)
                    nc.tensor.matmul(
                        sc[:, TM + q0:2 * TM], kt[:, ksl],
                        qt[:, p.h1, TM * p.m + q0:TM * (p.m + 1)],
                        start=True, stop=True)
                    if causal:
                        if q0 == 0:
                            nc.scalar.activation(pt[:, i, :], sc[:], EXP,
                                                 scale=SCALE, bias=bias_t[:, 12:13])
                        else:
                            nc.scalar.activation(
                                pt[:, i, q0:TM], sc[:, q0:TM], EXP,
                                scale=SCALE, bias=bias_t[:, 12:13])
                            nc.scalar.activation(
                                pt[:, i, TM + q0:2 * TM], sc[:, TM + q0:2 * TM],
                                EXP, scale=SCALE, bias=bias_t[:, 12:13])
                    else:
                        mk = pm.tile([128, TM], f32, tag="mk", name="mk")
                        nc.sync.dma_start(out=mk[:], in_=ap["maskT"][ksl, p.qsl])
                        for base in (0, TM):
                            nc.vector.scalar_tensor_tensor(
                                out=sc[:, base:base + TM],
                                in0=sc[:, base:base + TM],
                                scalar=SCALE, in1=mk[:],
                                op0=MULT, op1=ADD)
                        nc.scalar.activation(pt[:, i, :], sc[:], EXP,
                                             scale=1.0, bias=bias_t[:, 12:13])
                if causal:
                    jp = kk0 - (p.nk - 4)
                    if jp >= 0:
                        # diagonal pair: mask after exp. Even half: the usual
                        # [tri|1s|tri] window zeroes the k>q strip of both
                        # heads in one DVE op. Odd half: the window starts
                        # 128 earlier and its operand [0s|tri|1s|0s|tri] also
                        # zeroes the fully-future 128-wide strip.
                        w0 = 128 * jp
                        nc.vector.tensor_tensor(
                            out=pt[:, 0, w0:TM + w0 + 128],
                            in0=pt[:, 0, w0:TM + w0 + 128],
                            in1=dm2[:], op=MULT)
                        nc.vector.tensor_tensor(
                            out=pt[:, 1, w0:TM + w0 + 256],
                            in0=pt[:, 1, w0:TM + w0 + 256],
                            in1=dm3[:], op=MULT)
                return pt

            prev_c = [None]

            def drain_pair(p, kk0, pt, st, sp):
                q0 = p.q0p(kk0)
                if p.bf:
                    for i in range(2):
                        kk = kk0 + i
                        for ih in range(2):
                            rhs = pt[:, i, TM * ih + q0:TM * (ih + 1)]
                            nc.tensor.matmul(
                                p.rs[ih][0:1, q0:TM], ones1b[:], rhs,
                                start=st and i == 0, stop=sp and i == 1)
                            nc.tensor.matmul(
                                p.av[ih][:, q0:TM], vsb16[:, kk, :], rhs,
                                start=st and i == 0, stop=sp and i == 1)
                else:
                    vp = vsb[:, kk0:kk0 + 2, :]
                    for ih in range(2):
                        rhs = pt[:, :, TM * ih + q0:TM * (ih + 1)]
                        nc.tensor.matmul(p.rs[ih][:, q0:TM], ones2[:], rhs,
                                         start=st, stop=sp, perf_mode=DR)
                        nc.tensor.matmul(p.av[ih][:, q0:TM], vp, rhs,
                                         start=st, stop=sp, perf_mode=DR)
                if not sp:
                    return
                # pair complete: normalize chain (runs on DVE/GpSimd under
                # the next pair's score visits).
                for i, h in ((0, p.h0), (1, p.h1)):
                    inv = pb.tile([1, TM], f32, tag="inv", name="inv")
                    nc.vector.reciprocal_approx_fast(out=inv[:], in_=p.rs[i][0:1, :])
                    invb = pb.tile([128, TM], f32, tag=f"invb{i}",
                                   name=f"invb{i}")
                    nc.gpsimd.partition_broadcast(invb[:], inv[:])
                    nc.vector.tensor_tensor(
                        out=ao[h][:, p.qsl], in0=p.av[i][:], in1=invb[:],
                        op=MULT)
                if p.pair == 0 and prev_c[0] is not None:
                    phase_c(prev_c[0])
                if p.pair == 1:
                    prev_c[0] = p.m

            pending = []
            for m in MORD:
                for pair in range(NH // 2):
                    p = _Pair(m, pair)
                    # interleave the 2 diagonal pair-visits among the others
                    # so their DVE mask ops spread across the pair's span
                    # instead of bunching (and serializing) at its end
                    nd = max(p.nk - 4, 0)
                    diag = list(range(nd, p.nk, 2))
                    rest = list(range(0, nd, 2))
                    order = []
                    while diag or rest:
                        if rest:
                            order.append(rest.pop(0))
                        if rest:
                            order.append(rest.pop(0))
                        if diag:
                            order.append(diag.pop(0))
                    for idx, kk0 in enumerate(order):
                        pt = emit_scores_pair(p, kk0)
                        pending.append(
                            (p, kk0, pt, idx == 0, idx == len(order) - 1))
                        if len(pending) > 1:
                            drain_pair(*pending.pop(0))
            while pending:
                drain_pair(*pending.pop(0))
            phase_c(prev_c[0])


def _build(causal):
    nc = bacc.Bacc("TRN2", target_bir_lowering=False, debug=False, num_devices=8)
    with tile.TileContext(nc) as tc:
        _emit(nc, tc, causal)
    nc.compile()
    return nc


def _canonical_causal_mask():
    neg = np.float32(np.finfo(np.float32).min)
    m = np.where(np.tril(np.ones((T, T), dtype=bool)), np.float32(0.0), neg)
    return m.astype(np.float32)


def _to_fp8(x):
    return np.clip(x, -240.0, 240.0).astype(NP8)


def kernel(**inputs):
    global LAST_RESULTS
    hs = np.ascontiguousarray(np.asarray(inputs["hidden_states"], dtype=np.float32))
    cos = np.asarray(inputs["cos"], dtype=np.float32)
    sin = np.asarray(inputs["sin"], dtype=np.float32)
    mask = np.asarray(inputs["attention_mask"], dtype=np.float32)
    Wq = np.asarray(inputs["Wq"], dtype=np.float32)
    Wk = np.asarray(inputs["Wk"], dtype=np.float32)
    Wv = np.asarray(inputs["Wv"], dtype=np.float32)
    Wo = np.asarray(inputs["Wo"], dtype=np.float32)
    bq = np.asarray(inputs["bq"], dtype=np.float32)
    bk = np.asarray(inputs["bk"], dtype=np.float32)
    bv = np.asarray(inputs["bv"], dtype=np.float32)

    causal = bool(np.array_equal(mask[0, 0], _canonical_causal_mask()))

    key = (causal,)
    if key not in _cache:
        _cache[key] = _build(causal)
    nc = _cache[key]

    tri01f = (np.arange(128)[:, None] <= np.arange(128)[None, :])
    zerf = np.zeros((128, 128))
    dmask2f = np.concatenate(
        [tri01f, np.ones((128, TM - 128)), tri01f], axis=1).astype(np.float32)
    # odd-half operand: [zeros | tri | ones(256) | zeros | tri], window
    # starts 128 before the odd half's own diagonal strip
    dmask3f = np.concatenate(
        [zerf, tri01f, np.ones((128, 256)), zerf, tri01f], axis=1).astype(np.float32)
    ident = np.eye(128, dtype=np.float32)
    if not causal:
        maskT = np.ascontiguousarray(mask[0, 0].T)

    in_maps = []
    for c in range(8):
        b, g = divmod(c, 4)
        sl_q = slice(512 * g, 512 * (g + 1))
        sl_kv = slice(128 * g, 128 * (g + 1))
        sinT = np.ascontiguousarray(sin[b].T)  # [D, T]
        # row d<64: +sin[d+64] (consumed at base partition 0 writing rows 64:128)
        # row d>=64: -sin[d-64] (consumed at base partition 64 writing rows 0:64)
        sinTs = np.concatenate([sinT[64:128], -sinT[0:64]], axis=0)
        bias = np.zeros((128, 13), dtype=np.float32)
        bias[:, 0:4] = bq[sl_q].reshape(4, 128).T
        bias[:, 4] = bk[sl_kv]
        bias[:, 5] = bv[sl_kv]
        bias[:, 6:12] = bias[:, 0:6] * SW
        bias[:, 12] = ESHIFT
        # [p, k, c] / [m, p, k, c] layouts: each DMA descriptor covers a
        # contiguous run per partition.
        hsT = hs[b].T                                          # [HID, T]
        hsTr = np.ascontiguousarray(
            _to_fp8(hsT).reshape(NKT, 128, NM, TM).transpose(2, 1, 0, 3))
        hsTr16 = np.ascontiguousarray(
            hsT[:, 0:TM].astype(NP_IN).reshape(NKT, 128, TM).transpose(1, 0, 2))
        wqkvT = np.concatenate(
            [Wq[sl_q], Wk[sl_kv], Wv[sl_kv]], axis=0).T        # [HID, 768]
        wqkvTr = np.ascontiguousarray(
            _to_fp8(wqkvT * SW).reshape(NKT, 128, 768).transpose(1, 0, 2))
        wqkvTr16 = np.ascontiguousarray(
            wqkvT.astype(NP_IN).reshape(NKT, 128, 768).transpose(1, 0, 2))
        woT = Wo[:, sl_q].T.astype(NP_IN)                      # [512, HID]
        woTr = np.ascontiguousarray(woT.reshape(NH, 128, HID).transpose(1, 0, 2))
        m = {
            "hsTr": hsTr,
            "hsTr16": hsTr16,
            "wqkvTr": wqkvTr,
            "wqkvTr16": wqkvTr16,
            "woTr": woTr,
            "bias": bias,
            "cosT": np.ascontiguousarray(cos[b].T).astype(NP_IN),
            "sinTs": np.ascontiguousarray(sinTs).astype(NP_IN),
            "dmask2": dmask2f.astype(NP8),
            "dmask3": dmask3f.astype(NP8),
            "dmask2b": dmask2f.astype(NP_IN),
            "dmask3b": dmask3f.astype(NP_IN),
            "ident": ident.astype(NP_IN),
        }
        if not causal:
            m["maskT"] = maskT
        in_maps.append(m)

    trace = os.environ.get("KERNEL_TRACE", "0") == "1"
    res = run_bass_kernel_spmd(nc, in_maps, list(range(8)), trace=trace)
    LAST_RESULTS = res

    out = np.empty((B, T, HID), dtype=np.float32)
    for b in range(B):
        acc = np.asarray(res.results[4 * b]["out_part"], dtype=np.float32)
        for g in range(1, 4):
            acc = acc + np.asarray(
                res.results[4 * b + g]["out_part"], dtype=np.float32)
        out[b] = acc
    return out
